# revision 17
# baseline (speedup 1.0000x reference)
"""Trainium2 Bass kernel for the GRU caption model.

Computes: h0 = feat @ W_hp.T + b_hp; 200-step GRU whose hidden-proj gate
pre-activations are step-invariant; logits = outs @ W_out.T + b_out -> [B, V, T].

Strategy (v3):
- Every core runs the (tiny, latency-bound) GRU redundantly; the vocab dim of
  W_out/b_out is sharded 8 ways; each core emits a [3840, T, B] fp16 logits
  shard which the host transposes/concatenates.
- All recurrent state and 2-byte operands are fp16 (1 cyc/row matmuls, 2x/4x
  DVE modes, half DMA bytes); h0/gh are computed exactly in fp32, and the
  step-invariant gate constants are PRE-LOADED into PSUM so the matmuls
  accumulate on top (start=False), removing the "+C" adds from the chain.
- The gate PSUM is split into two tiles (r | z+n) so the r-gate tanh is only
  bank-serialized against the 16 r matmuls, not all 48 (PSUM deps are
  bank-level); sn2 goes to SBUF to dodge another bank serialization.
- Per step chain: r-mms -> tanh(r) [Act] -> mul, add [DVE] -> tanh(n) [Act]
  -> 2 fp16 tensor_tensor ops [DVE, 2x mode] -> h', written straight into the
  fp16 time-major state buffer read by both next-step matmuls and the vocab
  projection. z-gate products are computed off-chain on Pool.
- The vocab projection (bulk of PE work) is chopped into (<=16-step x 30
  vocab-tile) units and interleaved between GRU steps so PE stays dense; PSUM
  results are copied (+bias, fp16) to SBUF on Act/DVE alternately and written
  out with a few large fully-coalesced DMAs.
"""

import collections

import numpy as np
import ml_dtypes

import concourse.bass as bass
import concourse.mybir as mybir
import concourse.tile as tile
from concourse import bacc
from concourse.bass_utils import run_bass_kernel_spmd

F32 = mybir.dt.float32
F16 = mybir.dt.float16
AF = mybir.ActivationFunctionType
ALU = mybir.AluOpType

VOCAB = 30522
HID = 512
FEAT = 2048
STEPS = 200
B = 32
SOS = 101
NCORES = 8
P = 128
KO = HID // P           # 4 h-chunks
GM = 3 * HID // P       # 12 gate row-groups (r: 0-3, z: 4-7, n: 8-11)
KF = FEAT // P          # 16 feat chunks
VPAD = 3840             # per-core padded vocab rows = 30 * 128
MT = VPAD // P          # 30 vocab tiles per core

# projection column groups: small head groups (fill PE early), 16-step slabs,
# a 12-step tail group (shortens the post-loop drain)
GROUPS = ([(0, 4), (4, 12)] + [(t, t + 16) for t in range(12, 172, 16)]
          + [(172, 184), (184, 192), (192, 200)])
assert GROUPS[-1][1] == STEPS and all(b - a <= 16 for a, b in GROUPS)

LAST_RESULTS = None  # test harness introspection


def build():
    nc = bacc.Bacc("TRN2", target_bir_lowering=False, debug=False)

    FEATP = nc.dram_tensor("FEATP", [P, KF, B], F16, kind="ExternalInput")
    WHPP = nc.dram_tensor("WHPP", [P, KF, HID], F16, kind="ExternalInput")
    WHHP = nc.dram_tensor("WHHP", [P, KO, 3 * HID], F16, kind="ExternalInput")
    WIHP = nc.dram_tensor("WIHP", [P, KO, GM, P], F16, kind="ExternalInput")
    WOUTP = nc.dram_tensor("WOUTP", [P, KO, VPAD], F16, kind="ExternalInput")
    BIHP = nc.dram_tensor("BIHP", [P, GM], F32, kind="ExternalInput")
    BHHP = nc.dram_tensor("BHHP", [P, GM], F32, kind="ExternalInput")
    BHPP = nc.dram_tensor("BHPP", [P, KO], F32, kind="ExternalInput")
    BOUTP = nc.dram_tensor("BOUTP", [P, MT], F32, kind="ExternalInput")
    X0P = nc.dram_tensor("X0P", [P, KO, B], F16, kind="ExternalInput")
    OUT = nc.dram_tensor("OUT", [P, MT, STEPS, B], F16, kind="ExternalOutput")

    with tile.TileContext(nc) as tc:
        with (
            tc.tile_pool(name="const", bufs=1) as const,
            tc.tile_pool(name="stream", bufs=2) as stream,
            tc.tile_pool(name="step", bufs=3) as sp,
            tc.tile_pool(name="obp", bufs=2) as obp,
            tc.tile_pool(name="psg", bufs=2, space="PSUM") as psg,
            tc.tile_pool(name="psn", bufs=1, space="PSUM") as psn,
            tc.tile_pool(name="psp", bufs=3, space="PSUM") as psp,
        ):
            # ---- loads needed by the h0/gh phase first ----
            featT = const.tile([P, KF, B], F16, tag="featT")
            nc.sync.dma_start(featT[:], FEATP[:, :, :])
            bih_sb = const.tile([P, GM], F32, tag="bih")
            nc.sync.dma_start(bih_sb[:], BIHP[:, :])
            bhh_sb = const.tile([P, GM], F32, tag="bhh")
            nc.sync.dma_start(bhh_sb[:], BHHP[:, :])
            bhp_sb = const.tile([P, KO], F32, tag="bhp")
            nc.sync.dma_start(bhp_sb[:], BHPP[:, :])
            bout_sb = const.tile([P, MT], F32, tag="bout")
            nc.sync.dma_start(bout_sb[:], BOUTP[:, :])

            # recurrent state, time-major: resB[:, t+1] = h_t ; resB[:, 0] = x0
            resB = const.tile([P, STEPS + 1, KO, B], F16, tag="resB")
            nc.sync.dma_start(resB[:, 0, :, :], X0P[:, :, :])

            # ---- h0 = feat @ W_hp.T + b_hp (fp32, exact) ----
            # b_hp is PRE-LOADED into PSUM; all matmuls accumulate (start=False)
            ps_hf = psg.tile([P, 16, B], F32, tag="gr")
            ps_h = ps_hf[:, 0:KO, :]
            nc.vector.tensor_copy(
                ps_h, bhp_sb[:, :, None].to_broadcast((P, KO, B))
            )
            for kc in range(4):
                wchunk = stream.tile([P, 4, HID], F16, tag="wst")
                nc.sync.dma_start(wchunk[:], WHPP[:, 4 * kc:4 * kc + 4, :])
                for j in range(4):
                    kf = 4 * kc + j
                    for ko in range(KO):
                        nc.tensor.matmul(
                            ps_hf[:, ko, :],
                            wchunk[:, j, ko * P:(ko + 1) * P],
                            featT[:, kf, :],
                            start=False, stop=(kf == KF - 1),
                            skip_group_check=True,
                        )
            h0T = const.tile([P, KO, B], F32, tag="h0T")
            nc.scalar.copy(h0T[:], ps_h)
            h0hh = const.tile([P, KO, B], F16, tag="h0hh")
            nc.scalar.mul(h0hh[:], h0T[:], 0.5)
            h0h = const.tile([P, KO, B], F16, tag="h0h")
            nc.scalar.copy(h0h[:], h0T[:])

            # ---- gh = h0 @ W_hh.T + b_hh (fp32, exact; step-invariant) ----
            ps_gaf = psg.tile([P, 16, B], F32, tag="gz")   # gh groups 0..7 (r,z)
            ps_ga = ps_gaf[:, 0:8, :]
            nc.vector.tensor_copy(
                ps_ga, bhh_sb[:, 0:8, None].to_broadcast((P, 8, B))
            )
            ps_gbf = psg.tile([P, 16, B], F32, tag="gr")   # gh groups 8..11 (n)
            ps_gb = ps_gbf[:, 0:KO, :]
            nc.vector.tensor_copy(
                ps_gb, bhh_sb[:, 8:GM, None].to_broadcast((P, KO, B))
            )
            for kc in range(2):
                wchunk2 = stream.tile([P, 2, 3 * HID], F16, tag="wst")
                nc.sync.dma_start(wchunk2[:], WHHP[:, 2 * kc:2 * kc + 2, :])
                for j in range(2):
                    k = 2 * kc + j
                    for m in range(GM):
                        dst = ps_ga[:, m, :] if m < 8 else ps_gb[:, m - 8, :]
                        nc.tensor.matmul(
                            dst,
                            wchunk2[:, j, m * P:(m + 1) * P],
                            h0h[:, k, :],
                            start=False, stop=(k == KO - 1),
                            skip_group_check=True,
                        )
            # remaining resident weights (not needed until the loop / t>=4)
            wih = const.tile([P, KO, GM, P], F16, tag="wih")
            nc.sync.dma_start(wih[:], WIHP[:, :, :, :])
            wout = const.tile([P, KO, VPAD], F16, tag="wout")
            for c in range(4):
                vs = slice(c * (VPAD // 4), (c + 1) * (VPAD // 4))
                nc.sync.dma_start(wout[:, :, vs], WOUTP[:, :, vs])

            # Cpre_r = gh_r + b_ih_r                       (r PSUM preload)
            # Cpre_zn[0:4] = gh_z + b_ih_z                 (z PSUM preload)
            # Cpre_zn[4:8] = 0.5*gh_n + b_ih_n             (n PSUM preload; the
            #                0.5 comes from r = (1+tanh)/2 expansion)
            Cpre_r = const.tile([P, KO, B], F32, tag="Cpre_r")
            for m in range(4):
                nc.scalar.activation(
                    Cpre_r[:, m, :], ps_ga[:, m, :], AF.Identity,
                    bias=bih_sb[:, m, None], scale=1.0,
                )
            Cpre_zn = const.tile([P, 8, B], F32, tag="Cpre_zn")
            for m in range(4):
                nc.scalar.activation(
                    Cpre_zn[:, m, :], ps_ga[:, 4 + m, :], AF.Identity,
                    bias=bih_sb[:, 4 + m, None], scale=1.0,
                )
            hn2f = const.tile([P, KO, B], F32, tag="hn2f")
            nc.scalar.mul(hn2f[:], ps_gb, 0.5)
            for i in range(KO):
                nc.scalar.activation(
                    Cpre_zn[:, 4 + i, :], hn2f[:, i, :], AF.Identity,
                    bias=bih_sb[:, 8 + i, None], scale=1.0,
                )
            hn2h = const.tile([P, KO, B], F16, tag="hn2h")
            nc.vector.tensor_copy(hn2h[:], hn2f[:])

            # ---- interleaved GRU + vocab projection ----
            unit_q = collections.deque()
            ob_tiles = {}

            def emit_unit():
                g, m = unit_q.popleft()
                t0, t1 = GROUPS[g]
                ts = t1 - t0
                if m == 0:
                    ob_tiles[g] = obp.tile(
                        [P, 15, 16, B], F16, tag="ob", name=f"ob{g}a"
                    )
                    ob_tiles[g + 100] = obp.tile(
                        [P, 15, 16, B], F16, tag="ob", name=f"ob{g}b"
                    )
                ob = ob_tiles[g + (100 if m >= 15 else 0)]
                pp = psp.tile([P, 16, B], F32, tag="pp")
                ps = pp[:, :ts, :]
                for k in range(KO):
                    nc.tensor.matmul(
                        ps,
                        wout[:, k, m * P:(m + 1) * P],
                        resB[:, 1 + t0:1 + t1, k, :],
                        start=(k == 0), stop=(k == KO - 1),
                    )
                dst = ob[:, m % 15, :ts, :]
                if m % 2 == 0:
                    nc.scalar.activation(
                        dst, ps, AF.Identity, bias=bout_sb[:, m, None], scale=1.0
                    )
                else:
                    nc.vector.tensor_scalar_add(dst, ps, bout_sb[:, m, None])
                if m == 14 or m == MT - 1:
                    half = 0 if m == 14 else 1
                    nc.sync.dma_start(
                        OUT[:, 15 * half:15 * half + 15, t0:t1, :],
                        ob[:, :, :ts, :],
                    )

            gi = 0
            for t in range(STEPS):
                # ---- GRU step t: reads resB[:, t], writes resB[:, t+1] ----
                grf = psg.tile([P, 16, B], F32, tag="gr")    # r gates
                gr = grf[:, 0:KO, :]
                gzf = psg.tile([P, 16, B], F32, tag="gz")    # z gates
                gz = gzf[:, 0:KO, :]
                gnf = psn.tile([P, 16, B], F32, tag="gn")    # n gates
                gn = gnf[:, 0:KO, :]
                nc.scalar.copy(gr, Cpre_r[:])                # PSUM preloads
                nc.vector.tensor_copy(gz, Cpre_zn[:, 0:4, :])
                nc.vector.tensor_copy(gn, Cpre_zn[:, 4:8, :])
                for i, (dst_m, wm) in enumerate(
                    [(gr[:, m, :], m) for m in range(4)]             # r first
                    + [(gn[:, m - 8, :], m) for m in range(8, 12)]   # then n
                    + [(gz[:, m - 4, :], m) for m in range(4, 8)]    # z last
                ):
                    for k in range(KO):
                        nc.tensor.matmul(
                            dst_m, wih[:, k, wm, :], resB[:, t, k, :],
                            start=False, stop=(k == KO - 1),
                            skip_group_check=True,
                        )
                tr = sp.tile([P, KO, B], F16, tag="tr")
                nc.scalar.activation(tr[:], gr, AF.Tanh, scale=0.5)
                tz = sp.tile([P, KO, B], F16, tag="tz")
                nc.scalar.activation(tz[:], gz, AF.Tanh, scale=0.5)
                # off-chain z products on Pool:
                #   hm = 0.5 - 0.5*tz ; w0 = (0.5 + 0.5*tz) * h0  (as um*h0b)
                hm = sp.tile([P, KO, B], F16, tag="hm")
                nc.gpsimd.tensor_scalar(hm[:], tz[:], -0.5, 0.5, ALU.mult, ALU.add)
                um = sp.tile([P, KO, B], F16, tag="um")
                nc.gpsimd.tensor_scalar_add(um[:], tz[:], 1.0)
                w0t = sp.tile([P, KO, B], F16, tag="w0t")
                nc.gpsimd.tensor_mul(w0t[:], um[:], h0hh[:])
                # n-gate chain
                a = sp.tile([P, KO, B], F16, tag="a")
                nc.vector.tensor_mul(a[:], tr[:], hn2h[:])
                snb = sp.tile([P, KO, B], F32, tag="snb")
                nc.vector.tensor_add(snb[:], gn, a[:])
                nT = sp.tile([P, KO, B], F16, tag="nT")
                nc.scalar.activation(nT[:], snb[:], AF.Tanh, scale=1.0)
                # h' = hm*n + w0
                tm = sp.tile([P, KO, B], F16, tag="tm")
                nc.vector.tensor_mul(tm[:], hm[:], nT[:])
                nc.vector.tensor_add(resB[:, t + 1, :, :], tm[:], w0t[:])

                # ---- interleave projection work ----
                if gi < len(GROUPS) and GROUPS[gi][1] == t:
                    unit_q.extend((gi, m) for m in range(MT))
                    gi += 1
                drain = 3 if len(unit_q) > 80 else (2 if (t % 8 != 7 or len(unit_q) > 52) else 1)
                for _ in range(drain):
                    if unit_q:
                        emit_unit()

            while gi < len(GROUPS):
                unit_q.extend((gi, m) for m in range(MT))
                gi += 1
            while unit_q:
                emit_unit()

    nc.compile()
    return nc


def _shard_inputs(feat, W_hp, b_hp, W_ih, W_hh, b_ih, b_hh, embed, W_out, b_out):
    f16 = np.float16
    f32 = np.float32

    def pk(x, parts):  # [(k p), rest] -> [p, k, rest]
        x = np.asarray(x)
        return np.ascontiguousarray(
            x.reshape(parts, P, *x.shape[1:]).transpose(1, 0, *range(2, x.ndim + 1))
        )

    featP = pk(np.asarray(feat, f32).T, KF).astype(f16)         # [P, KF, B]
    whpP = pk(np.asarray(W_hp, f32).T, KF).astype(f16)          # [P, KF, HID]
    whhP = pk(np.asarray(W_hh, f32).T, KO).astype(f16)          # [P, KO, 3H]
    wihP = pk(np.asarray(W_ih, f32).T, KO).reshape(P, KO, GM, P).astype(f16)
    bihP = np.ascontiguousarray(np.asarray(b_ih, f32).reshape(GM, P).T)
    bhhP = np.ascontiguousarray(np.asarray(b_hh, f32).reshape(GM, P).T)
    bhpP = np.ascontiguousarray(np.asarray(b_hp, f32).reshape(KO, P).T)
    x0 = np.asarray(embed)[SOS].astype(f32).reshape(KO, P).T    # [P, KO]
    x0P = np.ascontiguousarray(
        np.repeat(x0[:, :, None], B, axis=2)
    ).astype(f16)                                               # [P, KO, B]

    Wo = np.zeros((NCORES * VPAD, HID), f32)
    Wo[:VOCAB] = W_out
    bo = np.zeros((NCORES * VPAD,), f32)
    bo[:VOCAB] = b_out
    common = dict(
        FEATP=featP, WHPP=whpP, WHHP=whhP, WIHP=wihP,
        BIHP=bihP, BHHP=bhhP, BHPP=bhpP, X0P=x0P,
    )
    in_maps = []
    for c in range(NCORES):
        sl = slice(c * VPAD, (c + 1) * VPAD)
        m = dict(common)
        m["WOUTP"] = pk(np.ascontiguousarray(Wo[sl].T), KO).astype(f16)
        m["BOUTP"] = np.ascontiguousarray(bo[sl].reshape(MT, P).T)
        in_maps.append(m)
    return in_maps


def kernel(**inputs):
    global LAST_RESULTS
    args = {k: np.asarray(v) for k, v in inputs.items()}
    in_maps = _shard_inputs(
        args["feat"], args["W_hp"], args["b_hp"], args["W_ih"], args["W_hh"],
        args["b_ih"], args["b_hh"], args["embed"], args["W_out"], args["b_out"],
    )
    nc = build()
    res = run_bass_kernel_spmd(nc, in_maps, core_ids=list(range(NCORES)))
    LAST_RESULTS = res
    # per-core OUT: [P, MT, T, B] fp16; vocab row = m*P + p
    shards = []
    for r in res.results:
        arr = np.asarray(r["OUT"])                      # [128, 30, 200, 32]
        shards.append(arr.transpose(3, 1, 0, 2).reshape(B, VPAD, STEPS))
    out = np.concatenate(shards, axis=1)[:, :VOCAB, :]
    return np.ascontiguousarray(out, dtype=np.float32)
